# revision 65
# baseline (speedup 1.0000x reference)
"""Trainium2 Bass kernel for nn_EquivariantNeuralField.

Per-pixel top-4-nearest-latent cross-attention neural field.
Sharding: 8 cores; core i handles batch i//4, pixel rows (i%4)*4096..+4096.

v2 design (vs baseline):
  - ALL scalar-engine activations live in one table set (silu_and_others):
    Sin (range-reduced), Silu (gelu via sigmoid-approx, scale folded into
    next-layer weights), Tanh (softmax exp via e^x=(1+tanh(x/2))/(1-tanh)),
    Copy/Identity drains.  -> zero LoadActFuncSet churn in steady state.
  - Distances on PE: nzx = [x0;x1;|x|^2]^T @ [2p0;2p1;-|p|^2] (fp32 matmul).
  - Gathers single-pass f32r (c,k tables); p/invg2 gathered pixel-major
    (f32r hi/lo 2-pass, 16 tiny matmuls) for exact rel positions.
  - Sin-embedding phase matmul fp32 once (q+v features fused, rel rhs).
  - Attention broadcast via gpsimd partition_broadcast (bf16, SBUF),
    att flattened through a small DRAM round-trip (4-way split DMAs).
"""
import numpy as np

B, N, L, K = 2, 16384, 256, 4
DIN, DOUT, DLAT, H, A, NH = 2, 3, 64, 128, 32, 4
NCORE = 8
NPC = N * B // NCORE          # pixels per core = 4096
CHUNK = 128
PI = float(np.pi)
SG = 1.702                    # gelu ~ x*sigmoid(SG*x) = silu(SG*x)/SG

_cache = {}


def _build(nchunk, debug=False):
    import concourse.bacc as bacc
    import concourse.mybir as mybir
    from concourse.tile import TileContext

    F32 = mybir.dt.float32
    F32R = mybir.dt.float32r
    BF16 = mybir.dt.bfloat16
    I32 = mybir.dt.int32
    U32 = mybir.dt.uint32
    AF = mybir.ActivationFunctionType
    OP = mybir.AluOpType

    nc = bacc.Bacc()

    # ---------------- DRAM tensors ----------------
    xd = nc.dram_tensor("x", [NPC, DIN], F32, kind="ExternalInput")
    pd = nc.dram_tensor("p", [L, DIN], F32, kind="ExternalInput")
    cd = nc.dram_tensor("c", [L, DLAT], F32, kind="ExternalInput")
    gd = nc.dram_tensor("g", [L, 1], F32, kind="ExternalInput")
    W_stem = nc.dram_tensor("W_stem", [DLAT, H], F32, kind="ExternalInput")
    b_stem = nc.dram_tensor("b_stem", [H], F32, kind="ExternalInput")
    Wq_sin = nc.dram_tensor("Wq_sin", [DIN, H // 2], F32, kind="ExternalInput")
    Wq1 = nc.dram_tensor("Wq1", [H + DIN, H], F32, kind="ExternalInput")
    bq1 = nc.dram_tensor("bq1", [H], F32, kind="ExternalInput")
    Wq2 = nc.dram_tensor("Wq2", [H, NH * A], F32, kind="ExternalInput")
    bq2 = nc.dram_tensor("bq2", [NH * A], F32, kind="ExternalInput")
    Wv_sin = nc.dram_tensor("Wv_sin", [DIN, H // 2], F32, kind="ExternalInput")
    Wv1 = nc.dram_tensor("Wv1", [H + DIN, H], F32, kind="ExternalInput")
    bv1 = nc.dram_tensor("bv1", [H], F32, kind="ExternalInput")
    Wv2 = nc.dram_tensor("Wv2", [H, 2 * H], F32, kind="ExternalInput")
    bv2 = nc.dram_tensor("bv2", [2 * H], F32, kind="ExternalInput")
    Wk = nc.dram_tensor("Wk", [H, NH * A], F32, kind="ExternalInput")
    bk = nc.dram_tensor("bk", [NH * A], F32, kind="ExternalInput")
    Wv = nc.dram_tensor("Wv", [H, NH * H], F32, kind="ExternalInput")
    bv = nc.dram_tensor("bv", [NH * H], F32, kind="ExternalInput")
    Wo1 = nc.dram_tensor("Wo1", [NH * H, NH * H], F32, kind="ExternalInput")
    bo1 = nc.dram_tensor("bo1", [NH * H], F32, kind="ExternalInput")
    Wo2 = nc.dram_tensor("Wo2", [NH * H, DOUT], F32, kind="ExternalInput")
    bo2 = nc.dram_tensor("bo2", [DOUT], F32, kind="ExternalInput")
    outd = nc.dram_tensor("out", [NPC, DOUT], F32, kind="ExternalOutput")
    dbg = {}
    if debug:
        for nm, shp in [("nzx", [128, 256]), ("m8", [128, 8]), ("idxf", [128, 8]),
                        ("c_kT", [128, 512]), ("k_kT", [128, 512]),
                        ("psm_pm", [128, 16]), ("rel_pm", [128, 8]),
                        ("sincc_pm", [128, 8]), ("te", [128, 512]),
                        ("fe", [128, 512]), ("S", [128, 512]), ("Ct", [128, 512]),
                        ("h1q", [128, 512]), ("h1v", [128, 512]),
                        ("qk", [128, 512]), ("u", [128, 512]),
                        ("lgpm", [128, 16]), ("att_pm", [128, 16]),
                        ("uw", [128, 2048]), ("y_bf", [128, 512]),
                        ("y1", [128, 512])]:
            dbg[nm] = nc.dram_tensor("dbg_" + nm, shp, F32, kind="ExternalOutput")

    with TileContext(nc) as tc:
        with tc.tile_pool(name="const", bufs=1) as cpool, \
             tc.tile_pool(name="work", bufs=2) as wpool, \
             tc.tile_pool(name="psA", bufs=2, space="PSUM") as psA, \
             tc.tile_pool(name="psY", bufs=2, space="PSUM") as psY, \
             tc.tile_pool(name="psS", bufs=3, space="PSUM") as psS, \
             tc.tile_pool(name="drp", bufs=2, space="DRAM") as drpool:

            # ============ one-time constants ============
            iota_i = cpool.tile([128, 256], I32)
            nc.gpsimd.iota(iota_i[:], [[1, 256]], base=0, channel_multiplier=0)
            iota_f = cpool.tile([128, 256], F32)
            nc.vector.tensor_copy(iota_f[:], iota_i[:])
            idn_i = cpool.tile([128, 128], I32)
            nc.gpsimd.iota(idn_i[:], [[1, 128]], base=0, channel_multiplier=-1)
            idn_f0 = cpool.tile([128, 128], F32)
            nc.vector.tensor_copy(idn_f0[:], idn_i[:])
            ident = cpool.tile([128, 128], F32)
            nc.vector.tensor_scalar(ident[:], idn_f0[:], 0.0, None, OP.is_equal)
            ident_bf = cpool.tile([128, 128], BF16)
            nc.vector.tensor_copy(ident_bf[:], ident[:])
            # blockones [128, NH] f32r : bo[c, h] = (c//A == h)
            blockones_f = cpool.tile([128, NH], F32)
            nc.gpsimd.memset(blockones_f[:], 0.0)
            for h in range(NH):
                nc.gpsimd.memset(blockones_f[h * A:(h + 1) * A, h:h + 1], 1.0)
            blockones = cpool.tile([128, NH], F32R)
            nc.vector.tensor_copy(blockones[:], blockones_f[:])
            halfpi = cpool.tile([128, 1], F32)
            nc.gpsimd.memset(halfpi[:], PI / 2.0)

            # ============ weights ============
            def load_cast(dram_ap, shape, dt, tag, scale=None):
                if dt == F32 and scale is None:
                    t0 = cpool.tile(shape, F32, tag=tag + "_f32", name=tag)
                    nc.sync.dma_start(t0[:], dram_ap)
                    return t0
                t0 = wpool.tile([128, 512], F32, tag="stage", name="stage_" + tag)
                nc.sync.dma_start(t0[0:shape[0], 0:shape[1]], dram_ap)
                t1 = cpool.tile(shape, dt, tag=tag)
                if scale is None:
                    nc.vector.tensor_copy(t1[:], t0[0:shape[0], 0:shape[1]])
                else:
                    nc.vector.tensor_scalar(t1[:], t0[0:shape[0], 0:shape[1]],
                                            float(scale), None, OP.mult)
                return t1

            def load_bias(dram, n, tag, scale=None):
                if n <= 128:
                    t = cpool.tile([n, 1], F32, tag=tag)
                    nc.sync.dma_start(t[:], dram[:].rearrange("(n o) -> n o", o=1))
                    if scale is not None:
                        nc.vector.tensor_scalar(t[:], t[:], float(scale), None, OP.mult)
                    return t
                k = n // 128
                t = cpool.tile([128, k], F32, tag=tag)
                nc.sync.dma_start(t[:], dram[:].rearrange("(j p) -> p j", p=128))
                if scale is not None:
                    nc.vector.tensor_scalar(t[:], t[:], float(scale), None, OP.mult)
                return t

            Wstem_t = load_cast(W_stem[:], [DLAT, H], F32, "wstem")
            Wq1_sin = load_cast(Wq1[DIN:DIN + 64, :], [64, H], F32R, "wq1sin")
            Wq1_cos = load_cast(Wq1[DIN + 64:DIN + 128, :], [64, H], F32R, "wq1cos")
            # Wv1 sin/cos live at partitions 64:128 (contract against S/Ct v-half)
            Wv1_sf = cpool.tile([128, H], F32, tag="wv1sf")
            nc.sync.dma_start(Wv1_sf[64:128, :], Wv1[DIN:DIN + 64, :])
            Wv1_sin_t = cpool.tile([128, H], F32R, tag="wv1sin")
            nc.vector.tensor_copy(Wv1_sin_t[64:128, :], Wv1_sf[64:128, :])
            Wv1_cf = cpool.tile([128, H], F32, tag="wv1cf")
            nc.sync.dma_start(Wv1_cf[64:128, :], Wv1[DIN + 64:DIN + 128, :])
            Wv1_cos_t = cpool.tile([128, H], F32R, tag="wv1cos")
            nc.vector.tensor_copy(Wv1_cos_t[64:128, :], Wv1_cf[64:128, :])
            Wv1_sin = Wv1_sin_t[64:128, :]
            Wv1_cos = Wv1_cos_t[64:128, :]
            # cc-rank2 weights [2,128] bf16
            def cc_w(dram_rows, tag):
                st = cpool.tile([2, 128], F32, tag=tag + "_f", name="st_" + tag)
                nc.sync.dma_start(st[:], dram_rows)
                t = cpool.tile([2, 128], BF16, tag=tag)
                nc.vector.tensor_copy(t[:], st[:])
                return t
            Wqcc_bf = cc_w(Wq1[0:DIN, :], "wqcc")
            Wvcc_bf = cc_w(Wv1[0:DIN, :], "wvcc")
            # silu scale folds: Wq2,Wv2,Wo2 get 1/SG
            Wq2_t = load_cast(Wq2[:], [H, NH * A], F32R, "wq2", scale=1.0 / SG)
            Wv2_t = load_cast(Wv2[:], [H, 2 * H], F32R, "wv2", scale=1.0 / SG)
            Wk_t = load_cast(Wk[:], [H, NH * A], F32, "wk")
            Wv_bf = load_cast(Wv[:], [H, NH * H], BF16, "wv")
            Wo1_f32 = cpool.tile([128, 4 * 512], F32, tag="wo1f")
            for c2 in range(4):
                nc.sync.dma_start(Wo1_f32[:, c2 * 512:(c2 + 1) * 512],
                                  Wo1[c2 * 128:(c2 + 1) * 128, :])
            Wo1_bf = cpool.tile([128, 4 * 512], BF16, tag="wo1")
            nc.vector.tensor_copy(Wo1_bf[:], Wo1_f32[:])
            Wo2_f32 = cpool.tile([128, 4 * DOUT], F32, tag="wo2f")
            for c2 in range(4):
                nc.sync.dma_start(Wo2_f32[:, c2 * DOUT:(c2 + 1) * DOUT],
                                  Wo2[c2 * 128:(c2 + 1) * 128, :])
            Wo2_bf = cpool.tile([128, 4 * DOUT], BF16, tag="wo2")
            nc.vector.tensor_scalar(Wo2_bf[:], Wo2_f32[:], 1.0 / SG, None, OP.mult)

            bstem_t = load_bias(b_stem, H, "bstem")
            bq1_t = load_bias(bq1, H, "bq1", scale=SG)
            bq2_t = load_bias(bq2, NH * A, "bq2")
            bv1_t = load_bias(bv1, H, "bv1", scale=SG)
            bv2_t = load_bias(bv2, 2 * H, "bv2")
            bk_t = load_bias(bk, NH * A, "bk")
            bv_t = load_bias(bv, NH * H, "bvt")
            bo1_t = load_bias(bo1, NH * H, "bo1")
            bo2_t = load_bias(bo2, DOUT, "bo2")

            # bo1' = SG*(bo1 + Wo1.T @ bv)  (fold attention bias + silu scale)
            bo1p = cpool.tile([128, 4], F32, tag="bo1p")
            bo1p_ps = psS.tile([128, 4], F32, tag="S", name="bo1p_s")
            for f2 in range(4):
                for c2 in range(4):
                    nc.tensor.matmul(
                        bo1p_ps[:, f2:f2 + 1],
                        Wo1_f32[:, c2 * 512 + f2 * 128:c2 * 512 + (f2 + 1) * 128],
                        bv_t[:, c2:c2 + 1],
                        start=(c2 == 0), stop=(c2 == 3))
            tmp_b = wpool.tile([128, 4], F32, tag="tmpb")
            nc.vector.tensor_tensor(tmp_b[:], bo1p_ps[:], bo1_t[:], OP.add)
            nc.vector.tensor_scalar(bo1p[:], tmp_b[:], SG, None, OP.mult)

            # sin-embedding weights [2,128] = pi*[Wq_sin|Wv_sin];
            # bias_e2 col = (W0+W1)/2 per feature.
            Wsin_st = cpool.tile([2, 128], F32, tag="wsin_st")
            nc.sync.dma_start(Wsin_st[:, 0:64], Wq_sin[:])
            nc.sync.dma_start(Wsin_st[:, 64:128], Wv_sin[:])
            Wsin_pi = cpool.tile([2, 128], F32, tag="wsin_pi")
            nc.vector.tensor_scalar(Wsin_pi[:], Wsin_st[:], PI, None, OP.mult)
            # bias_e2: te = (e_ps + pi*(W0+W1)) / (2pi); bias col = (W0+W1)/2
            WsinT_ps = psS.tile([128, 2], F32, tag="S", name="wsinT_s")
            nc.tensor.transpose(WsinT_ps[:], Wsin_st[:], ident[0:2, 0:2])
            bias_e2 = cpool.tile([128, 1], F32, tag="biase2")
            be_tmp = wpool.tile([128, 2], F32, tag="betmp")
            nc.vector.tensor_copy(be_tmp[:], WsinT_ps[:])
            # bias_e2 = (W0+W1)/2  (te = -e_ps/(2pi) + bias_e2)
            nc.vector.scalar_tensor_tensor(
                bias_e2[:], be_tmp[:, 0:1], 1.0, be_tmp[:, 1:2],
                OP.mult, OP.add)
            nc.vector.tensor_scalar(bias_e2[:], bias_e2[:], 0.5, None, OP.mult)

            # ============ latent tables (per batch/core) ============
            cT = cpool.tile([DLAT, L], F32, tag="cT")
            nc.sync.dma_start(cT[:], cd[:].rearrange("l d -> d l"))
            cstem_ps = psA.tile([128, 512], F32, tag="A", name="cstem_s")[:, 0:L]
            nc.tensor.matmul(cstem_ps[:], Wstem_t[:], cT[:], start=True, stop=True)
            cstemT = cpool.tile([128, L], F32, tag="cstemT")
            nc.scalar.activation(cstemT[:], cstem_ps[:], AF.Identity, bias=bstem_t[:, 0:1])

            c_tbl, k_tbl, psm_hi, psm_lo = [], [], [], []
            for lc in range(2):
                kl_ps = psA.tile([128, 512], F32, tag="A", name="kl_s")[:, 0:NH * A]
                nc.tensor.matmul(kl_ps[:], cstemT[:, lc * 128:(lc + 1) * 128],
                                 Wk_t[:], start=True, stop=True)
                kl = cpool.tile([128, NH * A], F32R, tag=f"kl{lc}")
                nc.vector.tensor_copy(kl[:], kl_ps[:])
                k_tbl.append(kl)
                cn_ps = psA.tile([128, 512], F32, tag="A", name="cn_s")[:, 0:128]
                nc.tensor.transpose(cn_ps[:], cstemT[:, lc * 128:(lc + 1) * 128], ident[:])
                cn = cpool.tile([128, 128], F32R, tag=f"cn{lc}")
                nc.vector.tensor_copy(cn[:], cn_ps[:])
                c_tbl.append(cn)
                # small table [128, 4] = (p0, p1, 1/g^2, 0), f32r hi/lo
                # (4 cols: 3-col f32r moving operand is invalid ISA)
                sm = cpool.tile([128, 4], F32, tag=f"smf{lc}")
                nc.gpsimd.memset(sm[:, 3:4], 0.0)
                nc.sync.dma_start(sm[:, 0:2], pd[lc * 128:(lc + 1) * 128, :])
                gt = wpool.tile([128, 1], F32, tag="gt", name=f"gt{lc}")
                nc.sync.dma_start(gt[:], gd[lc * 128:(lc + 1) * 128, :])
                g2 = wpool.tile([128, 1], F32, tag="g2", name=f"g2{lc}")
                nc.vector.tensor_tensor(g2[:], gt[:], gt[:], OP.mult)
                nc.vector.reciprocal(sm[:, 2:3], g2[:])
                shi_bf = wpool.tile([128, 4], BF16, tag="shibf", name=f"shibf{lc}")
                nc.vector.tensor_copy(shi_bf[:], sm[:])
                shi = cpool.tile([128, 4], F32R, tag=f"shi{lc}")
                nc.vector.tensor_copy(shi[:], shi_bf[:])
                slo = cpool.tile([128, 4], F32R, tag=f"slo{lc}")
                nc.vector.tensor_tensor(slo[:], sm[:], shi[:], OP.subtract)
                psm_hi.append(shi)
                psm_lo.append(slo)

            # paug [3, 256] f32: rows [-|p|^2; 2*p0; 2*p1], assembled via DMA
            # (DVE cannot write at unaligned partition bases).
            pT = cpool.tile([2, L], F32, tag="pT")
            nc.sync.dma_start(pT[:], pd[:].rearrange("l d -> d l"))
            p2 = wpool.tile([2, L], F32, tag="p2x")
            nc.vector.tensor_scalar(p2[:], pT[:], 2.0, None, OP.mult)
            psq = wpool.tile([2, L], F32, tag="psq")
            nc.vector.tensor_tensor(psq[:], pT[:], pT[:], OP.mult)
            ones2 = cpool.tile([2, 1], F32, tag="ones2")
            nc.gpsimd.memset(ones2[:], 1.0)
            psq_ps = psS.tile([1, 256], F32, tag="S", name="psq_s")
            nc.tensor.matmul(psq_ps[:], ones2[:], psq[:], start=True, stop=True)
            pn = wpool.tile([1, L], F32, tag="pn")
            nc.vector.tensor_scalar(pn[:], psq_ps[:], -1.0, None, OP.mult)
            paug = cpool.tile([3, L], F32, tag="paug")
            nc.sync.dma_start(paug[0:1, :], pn[:])
            nc.sync.dma_start(paug[1:3, :], p2[:])

            # ============ main loop over pixel chunks ============
            for ci in range(nchunk):
                n0 = ci * CHUNK
                # ---- A: distances on PE + top4 ----
                # nzx_mm = -|p|^2 + 2x.p = -|x-p|^2 + |x|^2 : the per-pixel
                # |x|^2 shift does not change the argmax; pen subtracts it.
                xaug = wpool.tile([3, 128], F32, tag="xaug", bufs=3)
                nc.gpsimd.memset(xaug[0:1, :], 1.0)
                nc.sync.dma_start(xaug[1:3, :], xd[n0:n0 + 128, :].rearrange("n c -> c n"))
                x0 = wpool.tile([128, 2], F32, tag="x0", bufs=3)
                nc.sync.dma_start(x0[:], xd[n0:n0 + 128, :])
                xsq = wpool.tile([128, 2], F32, tag="xsq", bufs=3)
                nc.vector.tensor_tensor(xsq[:], x0[:], x0[:], OP.mult)
                xn2 = wpool.tile([128, 1], F32, tag="xn2", bufs=3)
                nc.vector.tensor_reduce(xn2[:], xsq[:], mybir.AxisListType.X, OP.add)
                # Sa bank: nzx [:,0:256] + psm_pm [:,256:268]
                Sa = psS.tile([128, 512], F32, tag="S", name="Sa")
                nzx_ps = Sa[:, 0:256]
                psm_pm_ps = Sa[:, 256:272]
                nc.tensor.matmul(nzx_ps, xaug[:], paug[:], start=True, stop=True)
                m8 = wpool.tile([128, 8], F32, tag="m8", bufs=3)
                nc.vector.max(m8[:], nzx_ps)
                i8 = wpool.tile([128, 8], U32, tag="i8", bufs=3)
                nc.vector.max_index(i8[:], m8[:], nzx_ps)
                idxf = wpool.tile([128, 8], F32, tag="idxf", bufs=3)
                nc.vector.tensor_copy(idxf[:], i8[:])

                # ---- B: one-hots -> transposed -> gathers ----
                ohbig = psS.tile([128, 1024], BF16, tag="S", name="ohbig")
                for s in range(K):
                    oh = wpool.tile([128, 256], BF16, tag="oh", bufs=3)
                    nc.gpsimd.tensor_scalar(oh[:], iota_f[:], idxf[:, s:s + 1],
                                            None, OP.is_equal)
                    for lc in range(2):
                        nc.tensor.transpose(
                            ohbig[:, (lc * 4 + s) * 128:(lc * 4 + s + 1) * 128],
                            oh[:, lc * 128:(lc + 1) * 128], ident_bf[:])
                ohT = wpool.tile([128, 1024], F32R, tag="ohT", bufs=3)
                nc.scalar.copy(ohT[:, 0:512], ohbig[:, 0:512])
                nc.vector.tensor_copy(ohT[:, 512:1024], ohbig[:, 512:1024])

                ck_ps = psS.tile([128, 512], F32, tag="S", name="ck_ps")
                kk_ps = psS.tile([128, 512], F32, tag="S", name="kk_ps")
                for lc in range(2):
                    nc.tensor.matmul(ck_ps[:], c_tbl[lc][:],
                                     ohT[:, lc * 512:(lc + 1) * 512],
                                     start=(lc == 0), stop=(lc == 1))
                for lc in range(2):
                    nc.tensor.matmul(kk_ps[:], k_tbl[lc][:],
                                     ohT[:, lc * 512:(lc + 1) * 512],
                                     start=(lc == 0), stop=(lc == 1))
                # pixel-major gather of (p0,p1,invg2): [128, 12]
                for s in range(K):
                    first = True
                    for lc in range(2):
                        for tb in (psm_hi[lc], psm_lo[lc]):
                            nc.tensor.matmul(
                                psm_pm_ps[:, s * 4:(s + 1) * 4],
                                ohT[:, (lc * 4 + s) * 128:(lc * 4 + s + 1) * 128],
                                tb[:], start=first,
                                stop=(lc == 1 and tb is psm_lo[lc]),
                                skip_group_check=True)
                            first = False
                c_kT = wpool.tile([128, 512], F32, tag="c_kT", bufs=3)
                nc.scalar.copy(c_kT[:], ck_ps[:])
                k_kT = wpool.tile([128, 512], F32, tag="k_kT", bufs=3)
                nc.scalar.activation(k_kT[:], kk_ps[:], AF.Identity, bias=bk_t[:, 0:1])

                # ---- C: rel positions + sin features ----
                rel_pm = wpool.tile([128, 8], F32, tag="rel_pm", bufs=3)
                nc.vector.tensor_tensor(
                    rel_pm[:].rearrange("p (s c) -> p s c", c=2),
                    psm_pm_ps[:].rearrange("p (s c) -> p s c", c=4)[:, :, 0:2],
                    x0[:].rearrange("p (o c) -> p o c", o=1).to_broadcast([128, 4, 2]),
                    OP.subtract)
                # transpose rel to feature-major [2, 512] (per-s col blocks)
                Src = psS.tile([128, 512], F32, tag="S", name="Src")
                for s in range(K):
                    nc.tensor.transpose(Src[0:2, s * 128:(s + 1) * 128],
                                        rel_pm[:, 2 * s:2 * s + 2], ident[:])
                relT = wpool.tile([2, 512], F32, tag="relTsb", bufs=3)
                nc.vector.tensor_copy(relT[:], Src[0:2, :])
                # rel_pm = p - x; ref coords = x - p: sin(pi*(coords+1)) =
                # sin(pi*rel) = sin(2pi*wrap(rel/2)); floored-mod wrap in
                # feature-major: wm = (rel/2 + 0.5) mod 1; sin(2pi*wm - pi).
                # |pi*rel| <= 2pi: rely on HW sin wide-domain accuracy
                sincT = wpool.tile([2, 512], BF16, tag="sincTsb", bufs=3)
                nc.scalar.activation(sincT[:], relT[:], AF.Sin, scale=float(PI))

                # e phases: e_ps[:, s-blk] = Wsin_pi.T @ relT[:, s-blk]
                e_ps = psA.tile([128, 512], F32, tag="A")
                for s in range(K):
                    nc.tensor.matmul(e_ps[:, s * 128:(s + 1) * 128],
                                     Wsin_pi[:],
                                     relT[:, s * 128:(s + 1) * 128],
                                     start=True, stop=True)
                # e_ref = -e_ps + pi*(W0+W1): te = e_ref/(2pi)
                te = wpool.tile([128, 512], F32, tag="te", bufs=3)
                nc.scalar.activation(te[:], e_ps[:], AF.Identity,
                                     scale=float(-1.0 / (2 * PI)), bias=bias_e2[:, 0:1])
                ie = wpool.tile([128, 512], I32, tag="ie", bufs=3)
                nc.gpsimd.tensor_copy(ie[:], te[:])
                nfe32 = wpool.tile([128, 512], F32, tag="nfe32", bufs=3)
                nc.gpsimd.tensor_scalar(nfe32[:], ie[:], -1.0, None, OP.mult)
                fe = wpool.tile([128, 512], F32, tag="fe", bufs=3)
                nc.gpsimd.tensor_tensor(fe[:], te[:], nfe32[:], OP.add)
                fabs = wpool.tile([128, 512], F32, tag="fabs", bufs=3)
                nc.vector.scalar_tensor_tensor(fabs[:], fe[:], -1.0, fe[:],
                                               OP.mult, OP.max)
                S = wpool.tile([128, 512], F32R, tag="S", bufs=3)
                nc.scalar.activation(S[:], fe[:], AF.Sin, scale=float(2 * PI))
                Ct = wpool.tile([128, 512], F32R, tag="Ct", bufs=3)
                nc.scalar.activation(Ct[:], fabs[:], AF.Sin, scale=float(-2 * PI),
                                     bias=halfpi[:, 0:1])

                # ---- D: MLPs ----
                h1q_ps = psA.tile([128, 512], F32, tag="A")
                nc.tensor.matmul(h1q_ps[:], Wq1_sin[:], S[0:64, :],
                                 start=True, stop=False, skip_group_check=True)
                nc.tensor.matmul(h1q_ps[:], Wq1_cos[:], Ct[0:64, :],
                                 start=False, stop=False, skip_group_check=True)
                for s in range(K):
                    nc.tensor.matmul(h1q_ps[:, s * 128:(s + 1) * 128],
                                     Wqcc_bf[:],
                                     sincT[:, s * 128:(s + 1) * 128],
                                     start=False, stop=True, skip_group_check=True)
                h1q = wpool.tile([128, 512], F32R, tag="h1q", bufs=3)
                nc.scalar.activation(h1q[:], h1q_ps[:], AF.Silu, scale=SG,
                                     bias=bq1_t[:, 0:1])
                q_ps = psA.tile([128, 512], F32, tag="A")
                nc.tensor.matmul(q_ps[:], Wq2_t[:], h1q[:], start=True, stop=True)

                h1v_ps = psA.tile([128, 512], F32, tag="A")
                nc.tensor.matmul(h1v_ps[:], Wv1_sin, S[64:128, :],
                                 start=True, stop=False, skip_group_check=True)
                nc.tensor.matmul(h1v_ps[:], Wv1_cos, Ct[64:128, :],
                                 start=False, stop=False, skip_group_check=True)
                for s in range(K):
                    nc.tensor.matmul(h1v_ps[:, s * 128:(s + 1) * 128],
                                     Wvcc_bf[:],
                                     sincT[:, s * 128:(s + 1) * 128],
                                     start=False, stop=True, skip_group_check=True)
                h1v = wpool.tile([128, 512], F32R, tag="h1v", bufs=3)
                nc.scalar.activation(h1v[:], h1v_ps[:], AF.Silu, scale=SG,
                                     bias=bv1_t[:, 0:1])
                vg_ps = psA.tile([128, 512], F32, tag="A")
                nc.tensor.matmul(vg_ps[:], Wv2_t[:, 0:H], h1v[:], start=True, stop=True)
                vb_ps = psA.tile([128, 512], F32, tag="A")
                nc.tensor.matmul(vb_ps[:], Wv2_t[:, H:2 * H], h1v[:], start=True, stop=True)

                qk = wpool.tile([128, 512], F32R, tag="qk", bufs=3)
                nc.vector.scalar_tensor_tensor(qk[:], q_ps[:], bq2_t[:, 0:1], k_kT[:],
                                               OP.add, OP.mult)
                utmp = wpool.tile([128, 512], F32, tag="utmp", bufs=3)
                nc.vector.scalar_tensor_tensor(utmp[:], vg_ps[:], bv2_t[:, 0:1],
                                               c_kT[:], OP.add, OP.mult)
                u = wpool.tile([128, 512], BF16, tag="u", bufs=3)
                nc.vector.scalar_tensor_tensor(u[:], vb_ps[:], bv2_t[:, 1:2],
                                               utmp[:], OP.add, OP.add)

                # ---- E: logits + softmax (pixel-major), exp via tanh ----
                lg_tile = psA.tile([128, 512], F32, tag="A", name="lg_t")
                lg_ps = lg_tile[0:4, 0:512]
                nc.tensor.matmul(lg_ps, blockones[:], qk[:], start=True, stop=True)
                lg_sb = wpool.tile([4, 512], F32, tag="lg_sb", bufs=3)
                nc.scalar.copy(lg_sb[:], lg_ps)
                lgpm_t = psY.tile([128, 512], F32, tag="Y", name="lgpm_t")
                lgpm_ps = lgpm_t[:, 0:16]
                for s in range(K):
                    nc.tensor.transpose(lgpm_ps[:, s * 4:(s + 1) * 4],
                                        lg_sb[:, s * 128:(s + 1) * 128], ident[0:4, 0:4])
                # pen = invg2 * (m8_mm - |x|^2) = -invg2 * zx_k
                pen = wpool.tile([128, 4], F32, tag="pen", bufs=3)
                nc.vector.scalar_tensor_tensor(
                    pen[:], m8[:, 0:4], xn2[:, 0:1],
                    psm_pm_ps[:].rearrange("p (s c) -> p s c", c=4)[:, :, 2:3]
                    .rearrange("p s o -> p (s o)"),
                    OP.subtract, OP.mult)
                lgpm = wpool.tile([128, 16], F32, tag="lgpm", bufs=3)
                nc.vector.scalar_tensor_tensor(
                    lgpm[:].rearrange("p (s h) -> p s h", s=4),
                    lgpm_ps[:].rearrange("p (s h) -> p s h", s=4), 0.0,
                    pen[:].to_broadcast([128, 4, 4]), OP.add, OP.add)
                mx = wpool.tile([128, 4], F32, tag="mx", bufs=3)
                nc.vector.tensor_reduce(
                    mx[:], lgpm[:].rearrange("p (s h) -> p h s", s=4),
                    mybir.AxisListType.X, OP.max)
                esub = wpool.tile([128, 16], F32, tag="esub", bufs=3)
                nc.vector.tensor_tensor(
                    esub[:].rearrange("p (s h) -> p s h", s=4),
                    lgpm[:].rearrange("p (s h) -> p s h", s=4),
                    mx[:].rearrange("p (h o) -> p o h", o=1).to_broadcast([128, 4, 4]),
                    OP.subtract)
                # e^x = (1+t)/(1-t), t = tanh(x/2)
                th = wpool.tile([128, 16], F32, tag="th", bufs=3)
                nc.scalar.activation(th[:], esub[:], AF.Tanh, scale=0.5)
                den = wpool.tile([128, 16], F32, tag="den", bufs=3)
                nc.vector.tensor_scalar(den[:], th[:], -1.0, 1.0, OP.mult, OP.add)
                rden = wpool.tile([128, 16], F32, tag="rden", bufs=3)
                nc.vector.reciprocal(rden[:], den[:])
                epm = wpool.tile([128, 16], F32, tag="epm", bufs=3)
                nc.vector.scalar_tensor_tensor(epm[:], th[:], 1.0, rden[:],
                                               OP.add, OP.mult)
                zs = wpool.tile([128, 4], F32, tag="zs", bufs=3)
                nc.vector.tensor_reduce(
                    zs[:], epm[:].rearrange("p (s h) -> p h s", s=4),
                    mybir.AxisListType.X, OP.add)
                rz = wpool.tile([128, 4], F32, tag="rz", bufs=3)
                nc.vector.reciprocal(rz[:], zs[:])
                att_pm = wpool.tile([128, 16], F32, tag="att_pm", bufs=3)
                nc.vector.tensor_tensor(
                    att_pm[:].rearrange("p (h s) -> p s h", h=4),
                    epm[:].rearrange("p (s h) -> p s h", s=4),
                    rz[:].rearrange("p (h o) -> p o h", o=1).to_broadcast([128, 4, 4]),
                    OP.mult)
                attT_t = psY.tile([128, 512], F32, tag="Y", name="attT_t")
                nc.tensor.transpose(attT_t[0:16, 0:128], att_pm[:], ident[:])
                att_sh = wpool.tile([16, 128], BF16, tag="att_sh", bufs=3)
                nc.vector.tensor_copy(att_sh[:], attT_t[0:16, 0:128])
                att_dr = drpool.tile([16, 128], BF16, tag="att_dr")
                nc.sync.dma_start(att_dr[:], att_sh[:])
                att_flat = wpool.tile([1, 2048], BF16, tag="att_flat", bufs=2)
                for h in range(NH):
                    nc.sync.dma_start(
                        att_flat[0:1, h * 512:(h + 1) * 512],
                        att_dr[4 * h:4 * (h + 1), :].rearrange("r n -> (r n)")
                        .rearrange("(o f) -> o f", o=1))

                # ---- F: apply attention + output MLP ----
                uw = wpool.tile([128, 2048], BF16, tag="uw", bufs=3)
                for h in range(NH):
                    attB_sb = wpool.tile([128, 512], BF16, tag="attB_sb",
                                         name=f"attB{h}", bufs=2)
                    nc.gpsimd.partition_broadcast(
                        attB_sb[:], att_flat[0:1, h * 512:(h + 1) * 512])
                    nc.vector.tensor_tensor(
                        uw[:, h * 512:(h + 1) * 512], u[:], attB_sb[:], OP.mult)
                y_ps = psY.tile([128, 512], F32, tag="Y")
                for h in range(NH):
                    for s in range(K):
                        nc.tensor.matmul(
                            y_ps[:, h * 128:(h + 1) * 128],
                            Wv_bf[:, h * 128:(h + 1) * 128],
                            uw[:, h * 512 + s * 128:h * 512 + (s + 1) * 128],
                            start=(s == 0), stop=(s == 3))
                y_bf = wpool.tile([128, 512], BF16, tag="y_bf", bufs=3)
                nc.scalar.copy(y_bf[:], y_ps[:])
                y1_ps = psY.tile([128, 512], F32, tag="Y")
                for f2 in range(4):
                    for h in range(4):
                        nc.tensor.matmul(
                            y1_ps[:, f2 * 128:(f2 + 1) * 128],
                            Wo1_bf[:, h * 512 + f2 * 128:h * 512 + (f2 + 1) * 128],
                            y_bf[:, h * 128:(h + 1) * 128],
                            start=(h == 0), stop=(h == 3))
                y1 = wpool.tile([128, 512], BF16, tag="y1", bufs=3)
                for f2 in range(4):
                    nc.scalar.activation(y1[:, f2 * 128:(f2 + 1) * 128],
                                         y1_ps[:, f2 * 128:(f2 + 1) * 128],
                                         AF.Silu, scale=SG, bias=bo1p[:, f2:f2 + 1])
                o_tile = psY.tile([128, 512], F32, tag="Y", name="o_t")
                o_ps = o_tile[0:3, 0:128]
                for c2 in range(4):
                    nc.tensor.matmul(o_ps, Wo2_bf[:, c2 * 3:(c2 + 1) * 3],
                                     y1[:, c2 * 128:(c2 + 1) * 128],
                                     start=(c2 == 0), stop=(c2 == 3))
                o_sb = wpool.tile([3, 128], F32, tag="o_sb", bufs=3)
                nc.scalar.activation(o_sb[:], o_ps, AF.Identity, bias=bo2_t[:, 0:1])
                nc.sync.dma_start(outd[n0:n0 + 128, :].rearrange("n c -> c n"),
                                  o_sb[:])

                if debug and ci == 0:
                    nzx_sb = wpool.tile([128, 256], F32, tag="dbg_nzx")
                    nc.vector.tensor_copy(nzx_sb[:], nzx_ps[:])
                    psm_sb = wpool.tile([128, 16], F32, tag="dbg_psm")
                    nc.vector.tensor_copy(psm_sb[:], psm_pm_ps[:])
                    for nm, t in [("nzx", nzx_sb), ("m8", m8), ("idxf", idxf),
                                  ("c_kT", c_kT), ("k_kT", k_kT),
                                  ("psm_pm", psm_sb), ("rel_pm", rel_pm),
                                  ("te", te), ("fe", fe),
                                  ("S", S), ("Ct", Ct), ("h1q", h1q), ("h1v", h1v),
                                  ("qk", qk), ("u", u), ("lgpm", lgpm),
                                  ("att_pm", att_pm), ("uw", uw),
                                  ("y_bf", y_bf), ("y1", y1)]:
                        st = wpool.tile([128, 2048], F32, tag="dbgst",
                                        name="dbgst_" + nm)[0:t.shape[0], 0:t.shape[1]]
                        nc.vector.tensor_copy(st[:], t[:])
                        nc.sync.dma_start(dbg[nm][:], st[:])

    nc.compile()
    return nc


def kernel(**inputs):
    import jax
    try:
        jax.config.update('jax_platforms', 'axon,cpu')
    except Exception:
        pass
    from concourse.bass_utils import run_bass_kernel_spmd

    nchunk = NPC // CHUNK
    if nchunk not in _cache:
        _cache[nchunk] = _build(nchunk)
    nc = _cache[nchunk]

    x = np.asarray(inputs["x"], np.float32)
    wkeys = ["W_stem", "b_stem", "Wq_sin", "Wq1", "bq1", "Wq2", "bq2",
             "Wv_sin", "Wv1", "bv1", "Wv2", "bv2", "Wk", "bk", "Wv", "bv",
             "Wo1", "bo1", "Wo2", "bo2"]
    in_maps = []
    for core in range(NCORE):
        b = core // (NCORE // B)
        sh = (core % (NCORE // B))
        m = {k: np.ascontiguousarray(np.asarray(inputs[k], np.float32)) for k in wkeys}
        m["x"] = np.ascontiguousarray(x[b, sh * NPC:(sh + 1) * NPC])
        m["p"] = np.ascontiguousarray(np.asarray(inputs["p"], np.float32)[b])
        m["c"] = np.ascontiguousarray(np.asarray(inputs["c"], np.float32)[b])
        m["g"] = np.ascontiguousarray(np.asarray(inputs["g"], np.float32)[b])
        in_maps.append(m)

    res = run_bass_kernel_spmd(nc, in_maps, core_ids=list(range(NCORE)))
    out = np.zeros((B, N, DOUT), np.float32)
    for core in range(NCORE):
        b = core // (NCORE // B)
        sh = core % (NCORE // B)
        out[b, sh * NPC:(sh + 1) * NPC] = res.results[core]["out"]
    return out
